# revision 6
# baseline (speedup 1.0000x reference)
"""Trainium2 Bass kernel: fused multi-head attention (dense transformer block).

Reference computation (per batch element b of 8, one NeuronCore each):
    qkv = x @ w_qkv.T                  # [1024, 2304]
    q, k, v = split(qkv); reshape to 12 heads x 64 dims
    s = q @ k.T (unscaled); p = softmax(s); o = p @ v
    out = concat_heads(o) @ w_fc.T + b_fc

Kernel layout strategy (all per-core):
  - Everything runs in "transposed" layout: q_T/k_T are [head_dim, seq] so the
    TensorEngine can contract over head_dim directly; scores are computed as
    S_T[k, q] (keys on partitions) so exp needs no transpose and P_T feeds the
    P@V matmul as the moving operand.
  - Softmax skips max-subtraction (scores are bounded ~|70| < 88 overflow
    limit) and gets the denominator for free by appending a ones-column to V
    (M=65 output rows; row 64 = sum_k P).
  - Normalization happens on the small attention output (64x1024 per head):
    reciprocal of the denominator row, broadcast across partitions with a K=1
    matmul, one vector multiply.
  - The fc output is produced in natural [seq, dim] layout by using ao_T as
    the stationary operand, so no final transpose is needed.
  - Precision: qkv + scores run in float32r (TF32-like, ~1.6e-4 rel err,
    full PE speed); P, V, ao, w_fc in bf16. End-to-end ~4e-3 max rel err.
"""

import numpy as np
import concourse.bacc as bacc
import concourse.mybir as mybir
import concourse.tile as tile
from concourse.bass_utils import run_bass_kernel_spmd

SEQ = 1024
DIM = 768
H = 12
DH = 64
E = 3 * DIM  # 2304
NT = SEQ // 128  # 8  seq chunks
DT = DIM // 128  # 6  dim chunks
VA = H * (DH + 1)  # 780: v with ones column per head

f32 = mybir.dt.float32
f32r = mybir.dt.float32r
bf16 = mybir.dt.bfloat16
EXP = mybir.ActivationFunctionType.Exp


def build():
    nc = bacc.Bacc("TRN2", target_bir_lowering=False, debug=False)
    x_d = nc.dram_tensor("x", [SEQ, DIM], f32, kind="ExternalInput")
    wqkv_d = nc.dram_tensor("w_qkv", [E, DIM], f32, kind="ExternalInput")
    wfc_d = nc.dram_tensor("w_fc", [DIM, DIM], f32, kind="ExternalInput")
    bfc_d = nc.dram_tensor("b_fc", [1, DIM], f32, kind="ExternalInput")
    eye_d = nc.dram_tensor("eye", [128, 128], f32, kind="ExternalInput")
    out_d = nc.dram_tensor("out", [SEQ, DIM], f32, kind="ExternalOutput")

    with tile.TileContext(nc) as tc:
        with (
            tc.tile_pool(name="const", bufs=1) as constp,
            tc.tile_pool(name="persist", bufs=1) as persist,
        ):
            # ---- constants ----
            eye = constp.tile([128, 128], f32, tag="eye")
            nc.sync.dma_start(eye[:], eye_d.ap())
            ones_f = constp.tile([1, 128], f32, tag="onesf")
            nc.gpsimd.memset(ones_f[:], 1.0)
            ones_r = constp.tile([1, 128], f32r, tag="onesr")
            nc.vector.tensor_copy(ones_r[:], ones_f[:])
            bias_row = constp.tile([1, DIM], f32, tag="brow")
            nc.sync.dma_start(bias_row[:], bfc_d.ap())
            bias_r = constp.tile([1, DIM], f32r, tag="briasr")
            nc.vector.tensor_copy(bias_r[:], bias_row[:])
            bias_bc = constp.tile([128, DIM], f32, tag="bbc")

            # persistent across phases
            qkT = [persist.tile([128, SEQ], f32r, tag=f"qk{et}", name=f"qkT{et}") for et in range(12)]
            va = [persist.tile([128, VA], bf16, tag=f"va{nt}", name=f"va{nt}") for nt in range(NT)]
            aoT = [persist.tile([128, SEQ], bf16, tag=f"ao{j}", name=f"aoT{j}") for j in range(DT)]
            wfcT = [persist.tile([128, DIM], bf16, tag=f"wfcT{j}", name=f"wfcT{j}") for j in range(DT)]

            # ================= phase A/B: transposes + qkv/v matmuls ========
            with (
                tc.tile_pool(name="pab", bufs=1) as pab,
                tc.tile_pool(name="psT", bufs=1, space="PSUM") as psT,
                tc.tile_pool(name="psM", bufs=1, space="PSUM") as psM,
            ):
                xT = [pab.tile([128, SEQ], f32r, tag=f"xT{j}", name=f"xT{j}") for j in range(DT)]
                wqkvT = [pab.tile([128, E], f32r, tag=f"wT{j}", name=f"wqkvT{j}") for j in range(DT)]

                # bias broadcast to all 128 partitions via K=1 matmul
                bb_ps = psT.tile([128, DIM], f32, tag="tp", bufs=2)
                nc.tensor.matmul(bb_ps[:, 0:512], ones_r[:], bias_r[:, 0:512],
                                 start=True, stop=True)
                nc.tensor.matmul(bb_ps[:, 512:768], ones_r[:], bias_r[:, 512:768],
                                 start=True, stop=True)
                nc.any.tensor_copy(bias_bc[:], bb_ps[:])

                # x: load + PE-transpose into xT (f32r)
                for nt in range(NT):
                    xraw = pab.tile([128, DIM], f32, tag="xraw", bufs=3)
                    nc.sync.dma_start(xraw[:], x_d.ap()[nt * 128:(nt + 1) * 128, :])
                    for g, js in enumerate((range(0, 4), range(4, 6))):
                        tp = psT.tile([128, 128 * len(js)], f32, tag="tp", bufs=2)
                        for i, j in enumerate(js):
                            nc.tensor.transpose(
                                tp[:, i * 128:(i + 1) * 128],
                                xraw[:, j * 128:(j + 1) * 128], eye[:])
                        for i, j in enumerate(js):
                            nc.any.tensor_copy(
                                xT[j][:, nt * 128:(nt + 1) * 128],
                                tp[:, i * 128:(i + 1) * 128])

                # w_qkv: load + PE-transpose into wqkvT (f32r)
                for et in range(18):
                    wraw = pab.tile([128, DIM], f32, tag="wraw", bufs=3)
                    nc.sync.dma_start(wraw[:], wqkv_d.ap()[et * 128:(et + 1) * 128, :])
                    for g, js in enumerate((range(0, 4), range(4, 6))):
                        tp = psT.tile([128, 128 * len(js)], f32, tag="tp", bufs=2)
                        for i, j in enumerate(js):
                            nc.tensor.transpose(
                                tp[:, i * 128:(i + 1) * 128],
                                wraw[:, j * 128:(j + 1) * 128], eye[:])
                        for i, j in enumerate(js):
                            nc.any.tensor_copy(
                                wqkvT[j][:, et * 128:(et + 1) * 128],
                                tp[:, i * 128:(i + 1) * 128])

                # w_fc: load, cast bf16, xbar-transpose into wfcT
                for ft in range(DT):
                    fraw = pab.tile([128, DIM], f32, tag="wraw", bufs=3)
                    nc.sync.dma_start(fraw[:], wfc_d.ap()[ft * 128:(ft + 1) * 128, :])
                    fbf = pab.tile([128, DIM], bf16, tag="fbf", bufs=2)
                    nc.vector.tensor_copy(fbf[:], fraw[:])
                    for j in range(DT):
                        nc.sync.dma_start_transpose(
                            wfcT[j][:, ft * 128:(ft + 1) * 128],
                            fbf[:, j * 128:(j + 1) * 128])

                # qkv: q_T and k_T tiles [128e, 1024n], f32r
                for et in range(12):
                    ps = psM.tile([128, SEQ], f32, tag="mm", bufs=2)
                    for j in range(DT):
                        for h2 in range(2):
                            nc.tensor.matmul(
                                ps[:, h2 * 512:(h2 + 1) * 512],
                                wqkvT[j][:, et * 128:(et + 1) * 128],
                                xT[j][:, h2 * 512:(h2 + 1) * 512],
                                start=(j == 0), stop=(j == DT - 1))
                    nc.any.tensor_copy(qkT[et][:], ps[:])

                # v natural [128n, 12h x 64d] + ones column -> va (bf16)
                for nt in range(NT):
                    psv = psM.tile([128, DIM], f32, tag="mm", bufs=2)
                    for j in range(DT):
                        nc.tensor.matmul(psv[:, 0:512],
                                         xT[j][:, nt * 128:(nt + 1) * 128],
                                         wqkvT[j][:, 1536:2048],
                                         start=(j == 0), stop=(j == DT - 1))
                        nc.tensor.matmul(psv[:, 512:768],
                                         xT[j][:, nt * 128:(nt + 1) * 128],
                                         wqkvT[j][:, 2048:2304],
                                         start=(j == 0), stop=(j == DT - 1))
                    va3 = va[nt][:].rearrange("p (h c) -> p h c", c=DH + 1)
                    nc.gpsimd.memset(va3[:, :, DH:DH + 1], 1.0)
                    nc.any.tensor_copy(
                        va3[:, :, 0:DH],
                        psv[:].rearrange("p (h c) -> p h c", c=DH))

            # ================= phase C: attention per head pair =============
            with (
                tc.tile_pool(name="pc", bufs=1) as pc,
                tc.tile_pool(name="psS", bufs=1, space="PSUM") as psS,
                tc.tile_pool(name="psO", bufs=1, space="PSUM") as psO,
            ):
                for pair in range(6):
                    qt = qkT[pair]
                    kt = qkT[6 + pair]
                    PT = {}
                    for c in range(NT):
                        for xi in range(2):
                            ro = xi * 64
                            ps = psS.tile([128, SEQ], f32, tag="s", bufs=2)
                            for h2 in range(2):
                                nc.tensor.matmul(
                                    ps[:, h2 * 512:(h2 + 1) * 512],
                                    kt[ro:ro + 64, c * 128:(c + 1) * 128],
                                    qt[ro:ro + 64, h2 * 512:(h2 + 1) * 512],
                                    start=True, stop=True)
                            pt = pc.tile([128, SEQ], bf16, tag=f"pt{xi}_{c}", bufs=1)
                            nc.scalar.activation(pt[:], ps[:], EXP)
                            PT[(xi, c)] = pt
                    for xi in range(2):
                        hX = 2 * pair + xi
                        po = psO.tile([DH + 1, SEQ], f32, tag="o", bufs=2)
                        for c in range(NT):
                            va_h = va[c][:, hX * (DH + 1):(hX + 1) * (DH + 1)]
                            for h2 in range(2):
                                nc.tensor.matmul(
                                    po[:, h2 * 512:(h2 + 1) * 512],
                                    va_h,
                                    PT[(xi, c)][:, h2 * 512:(h2 + 1) * 512],
                                    start=(c == 0), stop=(c == NT - 1))
                        # normalize: recip of denominator row, broadcast, mul
                        recip = pc.tile([1, SEQ], f32r, tag="recip", bufs=2)
                        with nc.allow_low_precision(reason="f32r rounding"):
                            nc.vector.reciprocal(recip[:], po[DH:DH + 1, :])
                        bc_ps = psS.tile([64, SEQ], f32, tag="s", bufs=2)
                        for h2 in range(2):
                            nc.tensor.matmul(
                                bc_ps[:, h2 * 512:(h2 + 1) * 512],
                                ones_r[:, 0:64],
                                recip[:, h2 * 512:(h2 + 1) * 512],
                                start=True, stop=True)
                        bc_sb = pc.tile([64, SEQ], f32, tag="bc", bufs=2)
                        nc.scalar.copy(bc_sb[:], bc_ps[:])
                        nc.vector.tensor_mul(
                            aoT[pair][xi * 64:(xi + 1) * 64, :],
                            po[0:DH, :], bc_sb[:])

            # ================= phase D: fc + bias, natural layout ===========
            with (
                tc.tile_pool(name="pd", bufs=1) as pd,
                tc.tile_pool(name="psY", bufs=1, space="PSUM") as psY,
            ):
                for nt in range(NT):
                    psy = psY.tile([128, DIM], f32, tag="y", bufs=2)
                    for j in range(DT):
                        nc.tensor.matmul(psy[:, 0:512],
                                         aoT[j][:, nt * 128:(nt + 1) * 128],
                                         wfcT[j][:, 0:512],
                                         start=(j == 0), stop=(j == DT - 1))
                        nc.tensor.matmul(psy[:, 512:768],
                                         aoT[j][:, nt * 128:(nt + 1) * 128],
                                         wfcT[j][:, 512:768],
                                         start=(j == 0), stop=(j == DT - 1))
                    y = pd.tile([128, DIM], f32, tag="y_sb", bufs=3)
                    nc.vector.tensor_add(y[:], psy[:], bias_bc[:])
                    nc.sync.dma_start(out_d.ap()[nt * 128:(nt + 1) * 128, :], y[:])

    nc.compile()
    return nc


_NC = None
LAST_RESULTS = None  # BassKernelResults of the most recent run (for profiling)


def kernel(**inputs) -> np.ndarray:
    global _NC, LAST_RESULTS
    x = np.ascontiguousarray(np.asarray(inputs["x"], dtype=np.float32))
    w_qkv = np.ascontiguousarray(np.asarray(inputs["w_qkv"], dtype=np.float32))
    w_fc = np.ascontiguousarray(np.asarray(inputs["w_fc"], dtype=np.float32))
    b_fc = np.ascontiguousarray(
        np.asarray(inputs["b_fc"], dtype=np.float32).reshape(1, DIM))
    eye = np.eye(128, dtype=np.float32)

    if _NC is None:
        _NC = build()
    nc = _NC

    in_maps = [
        {"x": np.ascontiguousarray(x[b]), "w_qkv": w_qkv, "w_fc": w_fc,
         "b_fc": b_fc, "eye": eye}
        for b in range(8)
    ]
    res = run_bass_kernel_spmd(nc, in_maps, core_ids=list(range(8)))
    LAST_RESULTS = res
    out = np.stack([r["out"] for r in res.results], axis=0)
    return out.astype(np.float32)


if __name__ == "__main__":
    rng = np.random.default_rng(0)
    ins = {
        "x": rng.standard_normal((8, SEQ, DIM), dtype=np.float32),
        "w_qkv": (rng.standard_normal((E, DIM), dtype=np.float32) * DIM ** -0.5),
        "w_fc": (rng.standard_normal((DIM, DIM), dtype=np.float32) * DIM ** -0.5),
        "b_fc": (rng.standard_normal((DIM,), dtype=np.float32) * 0.02),
    }
    out = kernel(**ins)
    print("out", out.shape, out.dtype)


# revision 12
# speedup vs baseline: 1.2925x; 1.2925x over previous
"""Trainium2 Bass kernel: fused multi-head attention (dense transformer block).

Reference computation (per batch element b of 8, one NeuronCore each):
    qkv = x @ w_qkv.T                  # [1024, 2304]
    q, k, v = split(qkv); reshape to 12 heads x 64 dims
    s = q @ k.T (unscaled); p = softmax(s); o = p @ v
    out = concat_heads(o) @ w_fc.T + b_fc

Kernel layout strategy (all per-core):
  - Everything runs in "transposed" layout: q_T/k_T are [head_dim, seq] so the
    TensorEngine can contract over head_dim directly; scores are computed as
    S_T[k, q] (keys on partitions) so exp needs no transpose and P_T feeds the
    P@V matmul as the moving operand.
  - Softmax skips max-subtraction (scores are bounded ~|70| < 88 overflow
    limit) and gets the denominator for free by appending a ones-column to V
    (M=65 output rows; row 64 = sum_k P).
  - Normalization happens on the small attention output (64x1024 per head):
    reciprocal of the denominator row, broadcast across partitions with a K=1
    matmul, one vector multiply.
  - The fc output is produced in natural [seq, dim] layout by using ao_T as
    the stationary operand, so no final transpose is needed.
  - Precision: qkv + scores run in float32r (TF32-like, ~1.6e-4 rel err,
    full PE speed); P, V, ao, w_fc in bf16. End-to-end ~4e-3 max rel err.
"""

import numpy as np
import concourse.bacc as bacc
import concourse.mybir as mybir
import concourse.tile as tile
from concourse.bass_utils import run_bass_kernel_spmd

SEQ = 1024
DIM = 768
H = 12
DH = 64
E = 3 * DIM  # 2304
NT = SEQ // 128  # 8  seq chunks
DT = DIM // 128  # 6  dim chunks
VA = H * (DH + 1)  # 780: v with ones column per head

f32 = mybir.dt.float32
f32r = mybir.dt.float32r
bf16 = mybir.dt.bfloat16
EXP = mybir.ActivationFunctionType.Exp


def build():
    nc = bacc.Bacc("TRN2", target_bir_lowering=False, debug=False)
    x_d = nc.dram_tensor("x", [SEQ, DIM], f32, kind="ExternalInput")
    wqkv_d = nc.dram_tensor("w_qkv", [E, DIM], f32, kind="ExternalInput")
    wfc_d = nc.dram_tensor("w_fc", [DIM, DIM], f32, kind="ExternalInput")
    bfc_d = nc.dram_tensor("b_fc", [1, DIM], f32, kind="ExternalInput")
    eye_d = nc.dram_tensor("eye", [128, 128], f32, kind="ExternalInput")
    out_d = nc.dram_tensor("out", [SEQ, DIM], f32, kind="ExternalOutput")

    with tile.TileContext(nc) as tc:
        with (
            tc.tile_pool(name="const", bufs=1) as constp,
            tc.tile_pool(name="persist", bufs=1) as persist,
        ):
            # ---- constants ----
            eye = constp.tile([128, 128], f32, tag="eye")
            nc.sync.dma_start(eye[:], eye_d.ap())
            ones_f = constp.tile([1, 128], f32, tag="onesf")
            nc.gpsimd.memset(ones_f[:], 1.0)
            ones_r = constp.tile([1, 128], f32r, tag="onesr")
            nc.vector.tensor_copy(ones_r[:], ones_f[:])
            bias_row = constp.tile([1, DIM], f32, tag="brow")
            nc.sync.dma_start(bias_row[:], bfc_d.ap())
            bias_r = constp.tile([1, DIM], f32r, tag="briasr")
            nc.vector.tensor_copy(bias_r[:], bias_row[:])
            bias_bc = constp.tile([128, DIM], f32, tag="bbc")

            # persistent across phases
            qkT = [persist.tile([128, SEQ], f32r, tag=f"qk{et}", name=f"qkT{et}") for et in range(12)]
            va = [persist.tile([128, VA], bf16, tag=f"va{nt}", name=f"va{nt}") for nt in range(NT)]
            aoT = [persist.tile([128, SEQ], bf16, tag=f"ao{j}", name=f"aoT{j}") for j in range(DT)]
            wfcT = [persist.tile([128, DIM], bf16, tag=f"wfcT{j}", name=f"wfcT{j}") for j in range(DT)]

            # ================= phase A/B: transposes + qkv/v matmuls ========
            with (
                tc.tile_pool(name="pab", bufs=1) as pab,
                tc.tile_pool(name="psT", bufs=1, space="PSUM") as psT,
                tc.tile_pool(name="psM", bufs=1, space="PSUM") as psM,
            ):
                xT = [pab.tile([128, SEQ], f32r, tag=f"xT{j}", name=f"xT{j}") for j in range(DT)]
                wqkvT = [pab.tile([128, E], f32r, tag=f"wT{j}", name=f"wqkvT{j}") for j in range(DT)]

                # bias broadcast to all 128 partitions via K=1 matmul
                bb_ps = psT.tile([128, 512], f32, tag="tp", bufs=4)
                nc.tensor.matmul(bb_ps[:], ones_r[:], bias_r[:, 0:512],
                                 start=True, stop=True)
                nc.any.tensor_copy(bias_bc[:, 0:512], bb_ps[:])
                bb_ps2 = psT.tile([128, 256], f32, tag="tp", bufs=4)
                nc.tensor.matmul(bb_ps2[:], ones_r[:], bias_r[:, 512:768],
                                 start=True, stop=True)
                nc.any.tensor_copy(bias_bc[:, 512:768], bb_ps2[:])

                # x: load + PE-transpose into xT (f32r)
                for nt in range(NT):
                    xraw = pab.tile([128, DIM], f32, tag="xraw", bufs=3)
                    nc.sync.dma_start(xraw[:], x_d.ap()[nt * 128:(nt + 1) * 128, :])
                    for g, js in enumerate((range(0, 4), range(4, 6))):
                        tp = psT.tile([128, 128 * len(js)], f32, tag="tp", bufs=4)
                        for i, j in enumerate(js):
                            nc.tensor.transpose(
                                tp[:, i * 128:(i + 1) * 128],
                                xraw[:, j * 128:(j + 1) * 128], eye[:])
                        for i, j in enumerate(js):
                            nc.any.tensor_copy(
                                xT[j][:, nt * 128:(nt + 1) * 128],
                                tp[:, i * 128:(i + 1) * 128])

                # w_qkv: load + PE-transpose into wqkvT (f32r)
                for et in range(18):
                    wraw = pab.tile([128, DIM], f32, tag="wraw", bufs=3)
                    nc.sync.dma_start(wraw[:], wqkv_d.ap()[et * 128:(et + 1) * 128, :])
                    for g, js in enumerate((range(0, 4), range(4, 6))):
                        tp = psT.tile([128, 128 * len(js)], f32, tag="tp", bufs=4)
                        for i, j in enumerate(js):
                            nc.tensor.transpose(
                                tp[:, i * 128:(i + 1) * 128],
                                wraw[:, j * 128:(j + 1) * 128], eye[:])
                        for i, j in enumerate(js):
                            nc.any.tensor_copy(
                                wqkvT[j][:, et * 128:(et + 1) * 128],
                                tp[:, i * 128:(i + 1) * 128])

                # w_fc: load, cast bf16, xbar-transpose into wfcT
                for ft in range(DT):
                    fraw = pab.tile([128, DIM], f32, tag="wraw", bufs=3)
                    nc.sync.dma_start(fraw[:], wfc_d.ap()[ft * 128:(ft + 1) * 128, :])
                    fbf = pab.tile([128, DIM], bf16, tag="fbf", bufs=2)
                    nc.vector.tensor_copy(fbf[:], fraw[:])
                    for j in range(DT):
                        nc.sync.dma_start_transpose(
                            wfcT[j][:, ft * 128:(ft + 1) * 128],
                            fbf[:, j * 128:(j + 1) * 128])

                # qkv: q_T and k_T tiles [128e, 1024n], f32r
                for et in range(12):
                    ps = psM.tile([128, SEQ], f32, tag="mm", bufs=2)
                    for j in range(DT):
                        for h2 in range(2):
                            nc.tensor.matmul(
                                ps[:, h2 * 512:(h2 + 1) * 512],
                                wqkvT[j][:, et * 128:(et + 1) * 128],
                                xT[j][:, h2 * 512:(h2 + 1) * 512],
                                start=(j == 0), stop=(j == DT - 1))
                    nc.any.tensor_copy(qkT[et][:], ps[:])

                # v natural [128n, 12h x 64d] + ones column -> va (bf16)
                for nt in range(NT):
                    psv = psM.tile([128, DIM], f32, tag="mm", bufs=2)
                    for j in range(DT):
                        nc.tensor.matmul(psv[:, 0:512],
                                         xT[j][:, nt * 128:(nt + 1) * 128],
                                         wqkvT[j][:, 1536:2048],
                                         start=(j == 0), stop=(j == DT - 1))
                        nc.tensor.matmul(psv[:, 512:768],
                                         xT[j][:, nt * 128:(nt + 1) * 128],
                                         wqkvT[j][:, 2048:2304],
                                         start=(j == 0), stop=(j == DT - 1))
                    va3 = va[nt][:].rearrange("p (h c) -> p h c", c=DH + 1)
                    nc.gpsimd.memset(va3[:, :, DH:DH + 1], 1.0)
                    nc.any.tensor_copy(
                        va3[:, :, 0:DH],
                        psv[:].rearrange("p (h c) -> p h c", c=DH))

            # ================= phase C: attention per head pair =============
            with (
                tc.tile_pool(name="pc", bufs=1) as pc,
                tc.tile_pool(name="dsc", bufs=1, space="DRAM") as dscp,
                tc.tile_pool(name="psS", bufs=1, space="PSUM") as psS,
                tc.tile_pool(name="psO", bufs=1, space="PSUM") as psO,
            ):
                for pair in range(6):
                    qt = qkT[pair]
                    kt = qkT[6 + pair]
                    PT = {}
                    for c in range(NT):
                        for xi in range(2):
                            ro = xi * 64
                            ps = psS.tile([128, SEQ], f32, tag="s", bufs=2)
                            for h2 in range(2):
                                nc.tensor.matmul(
                                    ps[:, h2 * 512:(h2 + 1) * 512],
                                    kt[ro:ro + 64, c * 128:(c + 1) * 128],
                                    qt[ro:ro + 64, h2 * 512:(h2 + 1) * 512],
                                    start=True, stop=True)
                            pt = pc.tile([128, SEQ], bf16, tag=f"pt{xi}_{c}", bufs=1)
                            nc.scalar.activation(pt[:], ps[:], EXP)
                            PT[(xi, c)] = pt
                    for xi in range(2):
                        hX = 2 * pair + xi
                        po = psO.tile([DH + 1, SEQ], f32, tag="o", bufs=2)
                        for c in range(NT):
                            va_h = va[c][:, hX * (DH + 1):(hX + 1) * (DH + 1)]
                            for h2 in range(2):
                                nc.tensor.matmul(
                                    po[:, h2 * 512:(h2 + 1) * 512],
                                    va_h,
                                    PT[(xi, c)][:, h2 * 512:(h2 + 1) * 512],
                                    start=(c == 0), stop=(c == NT - 1))
                        # normalize: denom row -> [128,8] reshape -> wide recip
                        # -> DMA partition-broadcast -> multiply
                        drow = pc.tile([1, SEQ], f32, tag="drow", bufs=2)
                        nc.vector.tensor_copy(drow[:], po[DH:DH + 1, :])
                        dsc1 = dscp.tile([1, SEQ], f32, tag="dsc1", bufs=2)
                        nc.sync.dma_start(dsc1[:], drow[:])
                        den8 = pc.tile([128, 8], f32, tag="den8", bufs=2)
                        nc.sync.dma_start(
                            den8[:],
                            dsc1[:].rearrange("a (p c) -> (a p) c", c=8))
                        recip8 = pc.tile([128, 8], f32, tag="recip8", bufs=2)
                        nc.vector.reciprocal(recip8[:], den8[:])
                        dsc2 = dscp.tile([1, SEQ], f32, tag="dsc2", bufs=2)
                        nc.sync.dma_start(
                            dsc2[:].rearrange("a (p c) -> (a p) c", c=8),
                            recip8[:])
                        bc_sb = pc.tile([64, SEQ], f32, tag="bc", bufs=2)
                        nc.sync.dma_start(bc_sb[:],
                                          dsc2[:].broadcast_to([64, SEQ]))
                        nc.vector.tensor_mul(
                            aoT[pair][xi * 64:(xi + 1) * 64, :],
                            po[0:DH, :], bc_sb[:])

            # ================= phase D: fc + bias, natural layout ===========
            with (
                tc.tile_pool(name="pd", bufs=1) as pd,
                tc.tile_pool(name="psY", bufs=1, space="PSUM") as psY,
            ):
                for nt in range(NT):
                    psy = psY.tile([128, DIM], f32, tag="y", bufs=2)
                    for j in range(DT):
                        nc.tensor.matmul(psy[:, 0:512],
                                         aoT[j][:, nt * 128:(nt + 1) * 128],
                                         wfcT[j][:, 0:512],
                                         start=(j == 0), stop=(j == DT - 1))
                        nc.tensor.matmul(psy[:, 512:768],
                                         aoT[j][:, nt * 128:(nt + 1) * 128],
                                         wfcT[j][:, 512:768],
                                         start=(j == 0), stop=(j == DT - 1))
                    y = pd.tile([128, DIM], f32, tag="y_sb", bufs=3)
                    nc.vector.tensor_add(y[:], psy[:], bias_bc[:])
                    nc.sync.dma_start(out_d.ap()[nt * 128:(nt + 1) * 128, :], y[:])

    nc.compile()
    return nc


_NC = None
LAST_RESULTS = None  # BassKernelResults of the most recent run (for profiling)


def kernel(**inputs) -> np.ndarray:
    global _NC, LAST_RESULTS
    x = np.ascontiguousarray(np.asarray(inputs["x"], dtype=np.float32))
    w_qkv = np.ascontiguousarray(np.asarray(inputs["w_qkv"], dtype=np.float32))
    w_fc = np.ascontiguousarray(np.asarray(inputs["w_fc"], dtype=np.float32))
    b_fc = np.ascontiguousarray(
        np.asarray(inputs["b_fc"], dtype=np.float32).reshape(1, DIM))
    eye = np.eye(128, dtype=np.float32)

    if _NC is None:
        _NC = build()
    nc = _NC

    in_maps = [
        {"x": np.ascontiguousarray(x[b]), "w_qkv": w_qkv, "w_fc": w_fc,
         "b_fc": b_fc, "eye": eye}
        for b in range(8)
    ]
    res = run_bass_kernel_spmd(nc, in_maps, core_ids=list(range(8)))
    LAST_RESULTS = res
    out = np.stack([r["out"] for r in res.results], axis=0)
    return out.astype(np.float32)


if __name__ == "__main__":
    rng = np.random.default_rng(0)
    ins = {
        "x": rng.standard_normal((8, SEQ, DIM), dtype=np.float32),
        "w_qkv": (rng.standard_normal((E, DIM), dtype=np.float32) * DIM ** -0.5),
        "w_fc": (rng.standard_normal((DIM, DIM), dtype=np.float32) * DIM ** -0.5),
        "b_fc": (rng.standard_normal((DIM,), dtype=np.float32) * 0.02),
    }
    out = kernel(**ins)
    print("out", out.shape, out.dtype)
